# revision 1
# baseline (speedup 1.0000x reference)
"""Low-rank causal attention on 8 TRN2 NeuronCores.

Sharding: core c -> batch b = c//4, head-group hg = c%4 (4 of 16 heads).
Per-core kernel (no collectives):
  qkT = P(Wqk) @ x_b^T            [512, 2048]  (P = host row-permutation that
                                   places this core's q heads at partition
                                   stripes 32h and k heads likewise, so the
                                   K=16 attention matmuls row-group pack 4x)
  inv_q = 0.25/||q||, inv_k = 1/||k||  (full-rank norms via ones-matmul)
  v = x_b @ Wv_shard^T            [2048, 256] + ones column per head
  per (head, q-chunk 512, k-block 128):
     sT = kT_h^T-slice  x  qT_h   [128 nk, 512 nq]   (q pre-scaled by inv_q)
     pT = exp(inv_k[nk] * sT)      (ACT per-partition scale)
     pT *= mask01 (diagonal band blocks only)
     yT[h] += v_aug_h[kblk]^T-style matmul -> [65, 512] (row 64 = softmax denom)
Host unshard: y_head = (yT[0:64]/max(yT[64],1e-6)).T into out[b,:,head*64:+64].
"""

import os
from contextlib import ExitStack

import numpy as np
import ml_dtypes

import concourse.bass as bass
from concourse import bacc
import concourse.mybir as mybir
import concourse.tile as tile
from concourse.bass_utils import run_bass_kernel_spmd

B, N, D = 2, 2048, 1024
RANK, HEADS = 256, 16
HS = RANK // HEADS          # 16
DH = D // HEADS             # 64
NCORES = 8
HPC = 4                     # heads per core
QCH = 512                   # query chunk (free dim)
KB = 128                    # key block (partition dim)
NQC = N // QCH              # 4 query chunks
NKB = N // KB               # 16 key blocks
KTILES = D // 128           # 8 contraction tiles

F32 = mybir.dt.float32

_USE_BF16 = os.environ.get("KERNEL_DT", "bf16") == "bf16"
DT = mybir.dt.bfloat16 if _USE_BF16 else mybir.dt.float32
NPDT = ml_dtypes.bfloat16 if _USE_BF16 else np.float32

_CACHE = {}
LAST_RESULT = None


def _build_nc():
    nc = bacc.Bacc("TRN2", target_bir_lowering=False)
    xT = nc.declare_dram_parameter("xT", [D, N], DT, isOutput=False)
    wqkT = nc.declare_dram_parameter("wqkT", [D, 2 * RANK], DT, isOutput=False)
    wvT = nc.declare_dram_parameter("wvT", [D, HPC * DH], DT, isOutput=False)
    m01 = nc.declare_dram_parameter("m01", [NKB, KB, QCH], DT, isOutput=False)
    out = nc.declare_dram_parameter("out", [HPC * (DH + 1), N], F32, isOutput=True)

    with tile.TileContext(nc) as tc, ExitStack() as ctx:
        const = ctx.enter_context(tc.tile_pool(name="const", bufs=1))

        xT_sb = const.tile([128, KTILES, N], DT)
        wqkT_sb = const.tile([128, KTILES, 2 * RANK], DT)
        wvT_sb = const.tile([128, KTILES, HPC * DH], DT)
        for kk in range(KTILES):
            nc.sync.dma_start(xT_sb[:, kk, :], xT[128 * kk : 128 * kk + 128, :])
            nc.sync.dma_start(wqkT_sb[:, kk, :], wqkT[128 * kk : 128 * kk + 128, :])
            nc.sync.dma_start(wvT_sb[:, kk, :], wvT[128 * kk : 128 * kk + 128, :])

        ones_sb = const.tile([128, 128], F32)
        nc.vector.memset(ones_sb[:], 1.0)

        # v with an appended ones column per head: [nk-part, ntile, head, 65]
        v_sb = const.tile([128, NKB, HPC, DH + 1], DT)
        nc.vector.memset(v_sb[:, :, :, DH : DH + 1], 1.0)

        qT_sb = const.tile([128, N], DT)   # q rows (our heads at stripes 32h)
        kT_sb = const.tile([128, N], DT)   # k rows, unnormalized
        inv_kT = const.tile([128, NKB], F32)

        # ---------------- phase 1: projections + norms ----------------
        with (
            tc.tile_pool(name="qk_ps", bufs=4, space="PSUM") as qk_pool,
            tc.tile_pool(name="ss_ps", bufs=2, space="PSUM") as ss_pool,
            tc.tile_pool(name="v_ps", bufs=2, space="PSUM") as v_pool,
            tc.tile_pool(name="sq_sb", bufs=4) as sq_pool,
            tc.tile_pool(name="inv_sb", bufs=4) as inv_pool,
        ):
            for ci in range(NQC):
                ncol = slice(QCH * ci, QCH * ci + QCH)
                qk_ps = []
                for rt in range(4):
                    ps = qk_pool.tile([128, QCH], F32)
                    qk_ps.append(ps)
                    for kk in range(KTILES):
                        nc.tensor.matmul(
                            ps[:],
                            wqkT_sb[:, kk, 128 * rt : 128 * rt + 128],
                            xT_sb[:, kk, ncol],
                            start=(kk == 0),
                            stop=(kk == KTILES - 1),
                        )
                # sum of squares over all 256 q rows / 256 k rows,
                # replicated across all 128 partitions via ones-matmul
                sqs = []
                for rt in range(4):
                    sq = sq_pool.tile([128, QCH], F32, tag="sq")
                    nc.scalar.activation(
                        sq[:], qk_ps[rt][:], mybir.ActivationFunctionType.Square
                    )
                    sqs.append(sq)
                for half, scale in ((0, 16.0), (1, 1.0)):  # q: fold SCALE=0.25
                    ss = ss_pool.tile([128, QCH], F32)
                    nc.tensor.matmul(
                        ss[:], ones_sb[:], sqs[2 * half][:], start=True, stop=False
                    )
                    nc.tensor.matmul(
                        ss[:], ones_sb[:], sqs[2 * half + 1][:], start=False, stop=True
                    )
                    nrm = inv_pool.tile([128, QCH], F32, tag="nrm")
                    nc.scalar.activation(
                        nrm[:], ss[:], mybir.ActivationFunctionType.Sqrt, scale=scale
                    )
                    inv = inv_pool.tile([128, QCH], F32, tag="inv")
                    nc.vector.reciprocal(inv[:], nrm[:])
                    if half == 0:
                        # qT = q * (0.25/||q||), cast to DT
                        nc.vector.tensor_mul(qT_sb[:, ncol], qk_ps[0][:], inv[:])
                    else:
                        # k stays unnormalized; store 1/||k|| transposed
                        # inv row 0 holds the full chunk; one column per k-block
                        for jj in range(NQC):
                            nc.sync.dma_start(
                                inv_kT[:, NQC * ci + jj : NQC * ci + jj + 1],
                                inv[0:1, 128 * jj : 128 * jj + 128],
                            )
                nc.scalar.copy(kT_sb[:, ncol], qk_ps[2][:])

                # v projection for this chunk's 4 n-tiles
                for nt in range(NQC * ci, NQC * ci + NQC):
                    vp = v_pool.tile([128, HPC * DH], F32)
                    for kk in range(KTILES):
                        nc.tensor.matmul(
                            vp[:],
                            xT_sb[:, kk, 128 * nt : 128 * nt + 128],
                            wvT_sb[:, kk, :],
                            start=(kk == 0),
                            stop=(kk == KTILES - 1),
                        )
                    nc.scalar.copy(
                        v_sb[:, nt, :, 0:DH],
                        vp[:].rearrange("p (h e) -> p h e", h=HPC),
                    )

        # ---------------- phase 2: attention ----------------
        with (
            tc.tile_pool(name="st_ps", bufs=4, space="PSUM") as st_pool,
            tc.tile_pool(name="yt_ps", bufs=1, space="PSUM") as yt_pool,
            tc.tile_pool(name="pt_sb", bufs=6) as pt_pool,
            tc.tile_pool(name="m01_sb", bufs=3) as m01_pool,
            tc.tile_pool(name="yo_sb", bufs=4) as yo_pool,
        ):
            for ci in range(NQC):
                ncol = slice(QCH * ci, QCH * ci + QCH)
                nj = NQC * ci + NQC  # causal: k-blocks 0 .. 4*ci+3
                yts = [
                    yt_pool.tile([DH + 1, QCH], F32, name=f"yt{h}", tag=f"yt{h}")
                    for h in range(HPC)
                ]
                for j in range(nj):
                    band = j >= NQC * ci
                    if band:
                        mt = m01_pool.tile([128, QCH], DT)
                        nc.sync.dma_start(mt[:], m01[j])
                    for h in range(HPC):
                        st = st_pool.tile([128, QCH], F32)
                        nc.tensor.matmul(
                            st[:],
                            kT_sb[32 * h : 32 * h + HS, 128 * j : 128 * j + 128],
                            qT_sb[32 * h : 32 * h + HS, ncol],
                            start=True,
                            stop=True,
                            tile_position=(32 * h, 0),
                        )
                        pt = pt_pool.tile([128, QCH], DT)
                        nc.scalar.activation(
                            pt[:],
                            st[:],
                            mybir.ActivationFunctionType.Exp,
                            scale=inv_kT[:, j : j + 1],
                        )
                        if band:
                            nc.vector.tensor_mul(pt[:], pt[:], mt[:])
                        nc.tensor.matmul(
                            yts[h][:],
                            v_sb[:, j, h, :],
                            pt[:],
                            start=(j == 0),
                            stop=(j == nj - 1),
                        )
                for h in range(HPC):
                    yo = yo_pool.tile([DH + 1, QCH], F32, name=f"yo{h}", tag="yo")
                    nc.vector.tensor_copy(yo[:], yts[h][:])
                    nc.sync.dma_start(
                        out[(DH + 1) * h : (DH + 1) * (h + 1), ncol], yo[:]
                    )
    nc.compile()
    return nc


def _perm_for_core(hg: int) -> np.ndarray:
    """Row permutation of Wqk: this core's q heads land at partition stripes
    32h (h=0..3) of output r-tile 0, its k heads likewise in r-tile 2."""
    perm = np.empty(2 * RANK, dtype=np.int64)
    for part, base in ((0, 0), (1, RANK)):  # q rows then k rows
        ours = [HEADS * 0 + HPC * hg + h for h in range(HPC)]
        pos_used = np.zeros(RANK, dtype=bool)
        for h in range(HPC):
            head = HPC * hg + h
            rows = base + HS * head + np.arange(HS)
            perm[2 * RANK * 0 + base + 32 * h : base + 32 * h + HS] = rows
            pos_used[32 * h : 32 * h + HS] = True
        fill_rows = [
            base + HS * head + r
            for head in range(HEADS)
            if head not in range(HPC * hg, HPC * hg + HPC)
            for r in range(HS)
        ]
        fill_pos = np.flatnonzero(~pos_used)
        perm[base + fill_pos] = fill_rows
    return perm


def kernel(x, mask, Wqk, Wv):
    global LAST_RESULT
    x = np.asarray(x)
    mask = np.asarray(mask)
    Wqk = np.asarray(Wqk)
    Wv = np.asarray(Wv)

    if "nc" not in _CACHE:
        _CACHE["nc"] = _build_nc()
    nc = _CACHE["nc"]

    m01 = np.empty((NKB, KB, QCH), dtype=NPDT)
    for j in range(NKB):
        ci = j // NQC
        blk = mask[QCH * ci : QCH * ci + QCH, KB * j : KB * j + KB]
        m01[j] = (blk == 0).T.astype(NPDT)

    in_maps = []
    for c in range(NCORES):
        b, hg = divmod(c, HPC)
        perm = _perm_for_core(hg)
        in_maps.append(
            {
                "xT": np.ascontiguousarray(x[b].T).astype(NPDT),
                "wqkT": np.ascontiguousarray(Wqk[perm].T).astype(NPDT),
                "wvT": np.ascontiguousarray(
                    Wv[DH * HPC * hg : DH * HPC * (hg + 1)].T
                ).astype(NPDT),
                "m01": m01,
            }
        )

    trace = bool(os.environ.get("KBENCH_TRACE"))
    res = run_bass_kernel_spmd(nc, in_maps, list(range(NCORES)), trace=trace)
    LAST_RESULT = res

    y = np.empty((B, N, D), dtype=np.float32)
    for c in range(NCORES):
        b, hg = divmod(c, HPC)
        arr = res.results[c]["out"]
        for h in range(HPC):
            num = arr[(DH + 1) * h : (DH + 1) * h + DH]          # [64, N]
            den = np.maximum(arr[(DH + 1) * h + DH], 1e-6)       # [N]
            head = HPC * hg + h
            y[b, :, DH * head : DH * (head + 1)] = (num / den).T
    return y



# revision 5
# speedup vs baseline: 1.2280x; 1.2280x over previous
"""Low-rank causal attention on 8 TRN2 NeuronCores.

Sharding: core c -> batch b = c//4, head-group hg = c%4 (4 of 16 heads).
Per-core kernel (no collectives):
  qkT = P(Wqk) @ x_b^T            [512, 2048]  (P = host row-permutation that
                                   places this core's q heads at partition
                                   stripes 32h and k heads likewise, so the
                                   K=16 attention matmuls row-group pack 4x)
  qT = q * (0.25/||q||)  (rsqrt on replicated ssq), k'T = k * (1/||k||)
  v = x_b @ Wv_shard^T            [2048, 256] + ones column per head
  per (q-chunk 512, k-block 128):
     sT_h = k'T_h-slice x qT_h    [128 nk, 512 nq] for 4 heads (row-packed)
     heads 0-2: pT = exp(sT) in ONE merged ACT op [128, 1536]
     head 3:    pT = ((1+sT/4)*m)^4 on DVE (2nd-order exp approx, mask folded)
     band blocks: heads 0-2 multiplied by 0/1 mask on DVE
     yT[h] += v_aug_h[kblk]-stationary matmul -> [65, 512] (row 64 = denom)
  output DMA'd directly from PSUM.
Host unshard: y_head = (yT[0:64]/max(yT[64],1e-6)).T into out[b,:,head*64:+64].
"""

import os
from contextlib import ExitStack

import numpy as np
import ml_dtypes

import concourse.bass as bass
from concourse import bacc
import concourse.mybir as mybir
import concourse.tile as tile
from concourse.bass_utils import run_bass_kernel_spmd

B, N, D = 2, 2048, 1024
RANK, HEADS = 256, 16
HS = RANK // HEADS          # 16
DH = D // HEADS             # 64
NCORES = 8
HPC = 4                     # heads per core
QCH = 512                   # query chunk (free dim)
KB = 128                    # key block (partition dim)
NQC = N // QCH              # 4 query chunks
NKB = N // KB               # 16 key blocks
KTILES = D // 128           # 8 contraction tiles

F32 = mybir.dt.float32

_USE_BF16 = os.environ.get("KERNEL_DT", "bf16") == "bf16"
DT = mybir.dt.bfloat16 if _USE_BF16 else mybir.dt.float32
NPDT = ml_dtypes.bfloat16 if _USE_BF16 else np.float32

_CACHE = {}
LAST_RESULT = None


def _build_nc():
    nc = bacc.Bacc("TRN2", target_bir_lowering=False)
    xT = nc.declare_dram_parameter("xT", [D, N], DT, isOutput=False)
    wqkT = nc.declare_dram_parameter("wqkT", [D, 2 * RANK], DT, isOutput=False)
    wvT = nc.declare_dram_parameter("wvT", [D, HPC * DH], DT, isOutput=False)
    m01 = nc.declare_dram_parameter("m01", [NKB, KB, QCH], DT, isOutput=False)
    out = nc.declare_dram_parameter("out", [HPC * (DH + 1), N], F32, isOutput=True)

    with tile.TileContext(nc) as tc, ExitStack() as ctx:
        const = ctx.enter_context(tc.tile_pool(name="const", bufs=1))

        xT_sb = const.tile([128, KTILES, N], DT)
        wqkT_sb = const.tile([128, KTILES, 2 * RANK], DT)
        wvT_sb = const.tile([128, KTILES, HPC * DH], DT)
        for kk in range(KTILES):
            nc.sync.dma_start(xT_sb[:, kk, :], xT[128 * kk : 128 * kk + 128, :])
            nc.sync.dma_start(wqkT_sb[:, kk, :], wqkT[128 * kk : 128 * kk + 128, :])
            nc.sync.dma_start(wvT_sb[:, kk, :], wvT[128 * kk : 128 * kk + 128, :])

        ones_sb = const.tile([128, 128], DT)
        nc.vector.memset(ones_sb[:], 1.0)

        # v with an appended ones column per head: [nk-part, ntile, head, 65]
        v_sb = const.tile([128, NKB, HPC, DH + 1], DT)
        nc.vector.memset(v_sb[:, :, :, DH : DH + 1], 1.0)

        qT_sb = const.tile([128, N], DT)   # q rows pre-scaled by 0.25/||q||
        kT_sb = const.tile([128, N], DT)   # k rows pre-scaled by 1/||k||

        # ---------------- phase 1: projections + norms ----------------
        with (
            tc.tile_pool(name="qk_ps", bufs=4, space="PSUM") as qk_pool,
            tc.tile_pool(name="ss_ps", bufs=2, space="PSUM") as ss_pool,
            tc.tile_pool(name="v_ps", bufs=2, space="PSUM") as v_pool,
            tc.tile_pool(name="sq_sb", bufs=4) as sq_pool,
            tc.tile_pool(name="inv_sb", bufs=4) as inv_pool,
        ):
            for ci in range(NQC):
                ncol = slice(QCH * ci, QCH * ci + QCH)
                qk_ps = []
                for rt in range(4):
                    ps = qk_pool.tile([128, QCH], F32)
                    qk_ps.append(ps)
                    for kk in range(KTILES):
                        nc.tensor.matmul(
                            ps[:],
                            wqkT_sb[:, kk, 128 * rt : 128 * rt + 128],
                            xT_sb[:, kk, ncol],
                            start=(kk == 0),
                            stop=(kk == KTILES - 1),
                        )
                # sum of squares over all 256 q rows / 256 k rows,
                # replicated across all 128 partitions via ones-matmul
                sqs = []
                for rt in range(4):
                    sq = sq_pool.tile([128, QCH], DT, tag="sq")
                    nc.scalar.activation(
                        sq[:], qk_ps[rt][:], mybir.ActivationFunctionType.Square
                    )
                    sqs.append(sq)
                for half, scale in ((0, 16.0), (1, 1.0)):  # q: fold SCALE=0.25
                    ss = ss_pool.tile([128, QCH], F32)
                    nc.tensor.matmul(
                        ss[:], ones_sb[:], sqs[2 * half][:], start=True, stop=False
                    )
                    nc.tensor.matmul(
                        ss[:], ones_sb[:], sqs[2 * half + 1][:], start=False, stop=True
                    )
                    nrm = inv_pool.tile([128, QCH], F32, tag="nrm")
                    nc.scalar.activation(
                        nrm[:], ss[:], mybir.ActivationFunctionType.Sqrt, scale=scale
                    )
                    inv = inv_pool.tile([128, QCH], F32, tag="inv")
                    nc.vector.reciprocal_approx_fast(inv[:], nrm[:])
                    if half == 0:
                        # qT = q * (0.25/||q||), cast to DT
                        nc.vector.tensor_mul(qT_sb[:, ncol], qk_ps[0][:], inv[:])
                    else:
                        # kT = k * (1/||k||)
                        nc.vector.tensor_mul(kT_sb[:, ncol], qk_ps[2][:], inv[:])

                # v projection for this chunk's 4 n-tiles
                for nt in range(NQC * ci, NQC * ci + NQC):
                    vp = v_pool.tile([128, HPC * DH], F32)
                    for kk in range(KTILES):
                        nc.tensor.matmul(
                            vp[:],
                            xT_sb[:, kk, 128 * nt : 128 * nt + 128],
                            wvT_sb[:, kk, :],
                            start=(kk == 0),
                            stop=(kk == KTILES - 1),
                        )
                    nc.scalar.copy(
                        v_sb[:, nt, :, 0:DH],
                        vp[:].rearrange("p (h e) -> p h e", h=HPC),
                    )

        # ---------------- phase 2: attention ----------------
        with (
            tc.tile_pool(name="stA_ps", bufs=1, space="PSUM") as stA_pool,
            tc.tile_pool(name="stB_ps", bufs=1, space="PSUM") as stB_pool,
            tc.tile_pool(name="yt_ps", bufs=1, space="PSUM") as yt_pool,
            tc.tile_pool(name="pt_sb", bufs=3) as pt_pool,
            tc.tile_pool(name="u_sb", bufs=4) as u_pool,
            tc.tile_pool(name="m01_sb", bufs=3) as m01_pool,
            tc.tile_pool(name="yo_sb", bufs=4) as yo_pool,
        ):
            for ci in range(NQC):
                ncol = slice(QCH * ci, QCH * ci + QCH)
                nj = NQC * ci + NQC  # causal: k-blocks 0 .. 4*ci+3
                yts = [
                    yt_pool.tile([DH + 1, QCH], F32, name=f"yt{h}", tag=f"yt{h}")
                    for h in range(HPC)
                ]
                for j in range(nj):
                    band = j >= NQC * ci
                    if band:
                        mt = m01_pool.tile([128, QCH], DT)
                        nc.sync.dma_start(mt[:], m01[j])
                    stA = stA_pool.tile([128, 3, QCH], F32)
                    stB = stB_pool.tile([128, QCH], F32)
                    for h in range(3):
                        nc.tensor.matmul(
                            stA[:, h, :],
                            kT_sb[32 * h : 32 * h + HS, 128 * j : 128 * j + 128],
                            qT_sb[32 * h : 32 * h + HS, ncol],
                            start=True,
                            stop=True,
                            tile_position=(32 * h, 0),
                        )
                    nc.tensor.matmul(
                        stB[:],
                        kT_sb[96 : 96 + HS, 128 * j : 128 * j + 128],
                        qT_sb[96 : 96 + HS, ncol],
                        start=True,
                        stop=True,
                        tile_position=(96, 0),
                    )
                    pt = pt_pool.tile([128, HPC, QCH], DT)
                    # heads 0-2: one merged exp over [128, 1536]
                    nc.scalar.activation(
                        pt[:, 0:3, :], stA[:, :, :], mybir.ActivationFunctionType.Exp
                    )
                    # head 3: (1 + x/4)^4 ~ exp(x) (|x| <= 0.25), mask folded in
                    u = u_pool.tile([128, QCH], DT, tag="u")
                    nc.vector.tensor_scalar(
                        u[:], stB[:], 0.25, 1.0,
                        mybir.AluOpType.mult, mybir.AluOpType.add,
                    )
                    if band:
                        nc.vector.tensor_mul(u[:], u[:], mt[:])
                        for h in range(3):
                            nc.vector.tensor_mul(pt[:, h, :], pt[:, h, :], mt[:])
                    u2 = u_pool.tile([128, QCH], DT, tag="u2")
                    nc.vector.tensor_mul(u2[:], u[:], u[:])
                    nc.vector.tensor_mul(pt[:, 3, :], u2[:], u2[:])
                    for h in range(HPC):
                        nc.tensor.matmul(
                            yts[h][:],
                            v_sb[:, j, h, :],
                            pt[:, h, :],
                            start=(j == 0),
                            stop=(j == nj - 1),
                        )
                for h in range(HPC):
                    yo = yo_pool.tile([DH + 1, QCH], F32, name=f"yo{h}", tag="yo")
                    nc.any.tensor_copy(yo[:], yts[h][:])
                    nc.sync.dma_start(
                        out[(DH + 1) * h : (DH + 1) * (h + 1), ncol], yo[:]
                    )
    nc.compile()
    return nc


def _perm_for_core(hg: int) -> np.ndarray:
    """Row permutation of Wqk: this core's q heads land at partition stripes
    32h (h=0..3) of output r-tile 0, its k heads likewise in r-tile 2."""
    perm = np.empty(2 * RANK, dtype=np.int64)
    for part, base in ((0, 0), (1, RANK)):  # q rows then k rows
        pos_used = np.zeros(RANK, dtype=bool)
        for h in range(HPC):
            head = HPC * hg + h
            rows = base + HS * head + np.arange(HS)
            perm[base + 32 * h : base + 32 * h + HS] = rows
            pos_used[32 * h : 32 * h + HS] = True
        fill_rows = [
            base + HS * head + r
            for head in range(HEADS)
            if head not in range(HPC * hg, HPC * hg + HPC)
            for r in range(HS)
        ]
        fill_pos = np.flatnonzero(~pos_used)
        perm[base + fill_pos] = fill_rows
    return perm


def kernel(x, mask, Wqk, Wv):
    global LAST_RESULT
    x = np.asarray(x)
    mask = np.asarray(mask)
    Wqk = np.asarray(Wqk)
    Wv = np.asarray(Wv)

    if "nc" not in _CACHE:
        _CACHE["nc"] = _build_nc()
    nc = _CACHE["nc"]

    m01 = np.empty((NKB, KB, QCH), dtype=NPDT)
    for j in range(NKB):
        ci = j // NQC
        blk = mask[QCH * ci : QCH * ci + QCH, KB * j : KB * j + KB]
        m01[j] = (blk == 0).T.astype(NPDT)

    in_maps = []
    for c in range(NCORES):
        b, hg = divmod(c, HPC)
        perm = _perm_for_core(hg)
        in_maps.append(
            {
                "xT": np.ascontiguousarray(x[b].T).astype(NPDT),
                "wqkT": np.ascontiguousarray(Wqk[perm].T).astype(NPDT),
                "wvT": np.ascontiguousarray(
                    Wv[DH * HPC * hg : DH * HPC * (hg + 1)].T
                ).astype(NPDT),
                "m01": m01,
            }
        )

    trace = bool(os.environ.get("KBENCH_TRACE"))
    res = run_bass_kernel_spmd(nc, in_maps, list(range(NCORES)), trace=trace)
    LAST_RESULT = res

    y = np.empty((B, N, D), dtype=np.float32)
    for c in range(NCORES):
        b, hg = divmod(c, HPC)
        arr = res.results[c]["out"]
        for h in range(HPC):
            num = arr[(DH + 1) * h : (DH + 1) * h + DH]          # [64, N]
            den = np.maximum(arr[(DH + 1) * h + DH], 1e-6)       # [N]
            head = HPC * hg + h
            y[b, :, DH * head : DH * (head + 1)] = (num / den).T
    return y
